# revision 13
# baseline (speedup 1.0000x reference)
"""DBToAmplitude kernel for Trainium2: out = 10 ** features, elementwise.

features: (64, 80, 20000) float32.  Sharded batch-wise across 8 NeuronCores:
(8, 80, 20000) = 12.8M f32 elements per core.  Per core the flat stream is
viewed as [N_TILES, 128, F]; each tile is DMA'd HBM->SBUF, pushed through
the ScalarE activation LUT as Exp(ln(10) * x) (the affine scale is free),
then Newton-polished with one Ln pass to cancel the Exp table's ~1.1e-5
spline error (y = y0 * (1 + t - Ln(y0)), residual ~3e-6), and DMA'd back.
Memory-bound: ~102.4 MB of HBM traffic per core (~286us roofline at
358 GB/s); the 2 ACT passes (~167us) and 2 DVE ops (~209us) hide under it.
"""

import math
import time

import numpy as np

import concourse.bacc as bacc
import concourse.mybir as mybir
import concourse.tile as tile
from concourse.bass_utils import run_bass_kernel_spmd

N_CORES = 8
SHAPE = (64, 80, 20000)
TOTAL = SHAPE[0] * SHAPE[1] * SHAPE[2]          # 102,400,000
PER_CORE = TOTAL // N_CORES                     # 12,800,000
P = 128
FREE = PER_CORE // P                            # 100,000
F = 5000                                        # free-dim elements per tile
N_TILES = FREE // F                             # 20 tiles/core
LN10 = math.log(10.0)

VARIANT = "v5g"

_NC_CACHE = {}


def build_nc(variant=VARIANT, n_sweeps=1, f=F, bufs=(4, 4, 2), pool_mode="stack"):
    n_tiles = FREE // f
    assert n_tiles * f == FREE
    nc = bacc.Bacc("TRN2", target_bir_lowering=False, debug=False)
    x = nc.dram_tensor("x", [n_tiles, P, f], mybir.dt.float32, kind="ExternalInput")
    y = nc.dram_tensor("y", [n_tiles, P, f], mybir.dt.float32, kind="ExternalOutput")
    xap, yap = x.ap(), y.ap()
    mul = mybir.AluOpType.mult
    add = mybir.AluOpType.add
    sub = mybir.AluOpType.subtract
    with tile.TileContext(nc, pool_alloc_mode=pool_mode) as tc:
        with (
            tc.tile_pool(name="pin", bufs=bufs[0]) as pin,
            tc.tile_pool(name="py0", bufs=bufs[1]) as py0,
            tc.tile_pool(name="pl", bufs=bufs[2]) as pl,
        ):
            for _ in range(n_sweeps):
                for i in range(n_tiles):
                    tin = pin.tile([P, f], mybir.dt.float32)
                    if variant == "v5h":
                        load_eng = nc.sync if i % 2 == 0 else nc.scalar
                    elif variant == "v5m":
                        load_eng = nc.sync if i % 2 == 0 else nc.gpsimd
                    else:
                        load_eng = nc.sync
                    load_eng.dma_start(tin[:], xap[i][:])
                    y0 = py0.tile([P, f], mybir.dt.float32)
                    nc.scalar.activation(
                        y0[:], tin[:], mybir.ActivationFunctionType.Exp, scale=LN10
                    )
                    if variant == "v1":
                        nc.sync.dma_start(yap[i][:], y0[:])
                        continue
                    # l = Ln(y0); d = (tin*ln10 - l) over l's tile;
                    # y = (d + 1) * y0 over y0's tile.
                    l = pl.tile([P, f], mybir.dt.float32)
                    if variant == "v5exp":  # timing probe: Ln->Exp, same cost shape
                        nc.scalar.activation(
                            l[:], y0[:], mybir.ActivationFunctionType.Exp, scale=0.1
                        )
                    else:
                        nc.scalar.activation(
                            l[:], y0[:], mybir.ActivationFunctionType.Ln
                        )
                    nc.vector.scalar_tensor_tensor(
                        l[:], tin[:], LN10, l[:], op0=mul, op1=sub
                    )
                    nc.vector.scalar_tensor_tensor(
                        y0[:], l[:], 1.0, y0[:], op0=add, op1=mul
                    )
                    if variant in ("v5g", "v5h"):
                        nc.gpsimd.dma_start(yap[i][:], y0[:])
                    elif variant == "v5a":
                        nc.scalar.dma_start(yap[i][:], y0[:])
                    elif variant == "v5m":
                        store_eng = nc.gpsimd if i % 2 == 0 else nc.sync
                        store_eng.dma_start(yap[i][:], y0[:])
                    else:
                        nc.sync.dma_start(yap[i][:], y0[:])
    nc.compile()
    return nc


def _get_nc():
    if "nc" not in _NC_CACHE:
        _NC_CACHE["nc"] = build_nc()
    return _NC_CACHE["nc"]


def kernel(features: np.ndarray) -> np.ndarray:
    feats = np.ascontiguousarray(features, dtype=np.float32)
    shards = feats.reshape(N_CORES, N_TILES, P, F)
    in_maps = [{"x": shards[c]} for c in range(N_CORES)]
    last_err = None
    for attempt in range(4):
        try:
            res = run_bass_kernel_spmd(
                _get_nc(), in_maps, core_ids=list(range(N_CORES))
            )
            break
        except Exception as e:  # transient NRT_EXEC_UNIT_UNRECOVERABLE etc.
            last_err = e
            _NC_CACHE.clear()
            time.sleep(10 * (attempt + 1))
            try:
                import jax
                from jax.extend import backend as _jex_backend

                jax.clear_caches()
                _jex_backend.clear_backends()
            except Exception:
                pass
    else:
        raise last_err
    out = np.stack([res.results[c]["y"] for c in range(N_CORES)])
    return out.reshape(SHAPE)
